# revision 32
# baseline (speedup 1.0000x reference)
"""Baichuan paged-attention layer on 8 trn2 cores, tensor-parallel over heads.

Per core c: heads 4c..4c+3, all matmul math in bf16 (f32 PSUM accumulate).
Two passes of 2 sequences each: per pass the QK weights are loaded once and
the 1024-token hidden chunk is resident in SBUF, so QK weights are streamed
2x total (vs 4x per-seq in the f32 baseline). RoPE runs in f32 off PSUM and
rounds once to bf16. Attention (scores, exp, pv) is bf16 against [gathered
history KV + new KV]. o_proj stages [128,4096] row-blocks and writes bf16
partials; the host sums the 8 partials in f64.
"""
import sys

sys.path.insert(0, "/opt/trn_rl_repo")
import numpy as np
import ml_dtypes

BF16NP = ml_dtypes.bfloat16

H = 32; D = 128; HID = 4096; BS = 64; NBLOCKS = 128
B = 4; QLEN = 512; MAXBLK = 24; ROPE_BASE = 10000.0
T = B * QLEN; NCORES = 8; HC = H // NCORES; W = HC * D  # 4 heads, 512 wide
NEG = -1.0e30
SCALE = 1.0 / float(np.sqrt(D))

_cache = {}
last_results = None  # BassKernelResults of the most recent run (for test.py)


def _round128(x):
    return (x + 127) // 128 * 128


def _build(hist, reps=1):
    import concourse.bass as bass
    import concourse.tile as tile
    from concourse import bacc, mybir

    F32 = mybir.dt.float32
    F32R = mybir.dt.float32r
    BF16 = mybir.dt.bfloat16

    hv = [_round128(h) for h in hist]
    SH = [x // 128 for x in hv]

    nc = bacc.Bacc("TRN2", target_bir_lowering=False, debug=False,
                   num_devices=NCORES)
    hiddenT_d = nc.dram_tensor("hiddenT", [HID, T], BF16, kind="ExternalInput")
    # wqr: [rt, kslab, p, col] flattened -> [8*HID, 128]
    wqr_d = nc.dram_tensor("wqr", [8 * HID, 128], BF16, kind="ExternalInput")
    wvT_d = nc.dram_tensor("wvT", [HID, W], BF16, kind="ExternalInput")
    woT_d = nc.dram_tensor("woT", [W, HID], BF16, kind="ExternalInput")
    kh_d = [nc.dram_tensor(f"khT{b}", [W, hv[b]], BF16, kind="ExternalInput")
            if hv[b] else None for b in range(B)]
    vh_d = [nc.dram_tensor(f"vh{b}", [hv[b], W], BF16, kind="ExternalInput")
            if hv[b] else None for b in range(B)]
    out_d = nc.dram_tensor("out", [T, HID], BF16, kind="ExternalOutput")

    # host-built tables baked into the NEFF
    inv = 1.0 / (ROPE_BASE ** (np.arange(0, D, 2) / D))
    pos = np.concatenate([h + np.arange(QLEN) for h in hist]).astype(np.float64)
    ang = np.concatenate([inv, inv])[:, None] * pos[None, :]
    cos_d = nc.inline_tensor(np.cos(ang).astype(np.float32), name="cosT")
    sin_d = nc.inline_tensor(np.sin(ang).astype(np.float32), name="sinT")

    mask_np = np.where(
        np.arange(128)[:, None] <= np.arange(896)[None, :] - 384,
        0.0, NEG).astype(np.float32)
    mask_d = nc.inline_tensor(mask_np, name="maskS")

    pad_np = np.zeros((128, B), np.float32)
    for b in range(B):
        if hv[b]:
            pad_np[:, b] = np.where(hv[b] - 128 + np.arange(128) >= hist[b],
                                    NEG, 0.0)
    pad_d = nc.inline_tensor(pad_np, name="padc")

    Pm = np.zeros((128, 128), np.float32)
    for d in range(64):
        Pm[d, d + 64] = -1.0
        Pm[d + 64, d] = 1.0
    pt_d = nc.inline_tensor(np.ascontiguousarray(Pm.T), name="permT")
    ones_d = nc.inline_tensor(np.ones((128, 1), BF16NP), name="ones")

    with tile.TileContext(nc) as tc:
        with tc.tile_pool(name="const", bufs=1) as cpool, \
             tc.tile_pool(name="attn", bufs=16) as apool, \
             tc.tile_pool(name="psum", bufs=8, space="PSUM") as pspool:
            cs_loaded = []

            def _load_cs():
                # deferred so the first hid/wq DMAs win the HWDGE FIFO race
                mask_t = cpool.tile([128, 896], F32, tag="mask")
                nc.sync.dma_start(mask_t[:], mask_d[:])
                pad_t = cpool.tile([128, B], F32, tag="pad")
                nc.sync.dma_start(pad_t[:], pad_d[:])
                pt_t = cpool.tile([128, 128], F32R, tag="pt")
                nc.sync.dma_start(pt_t[:], pt_d[:].bitcast(F32R))
                ones_t = cpool.tile([128, 1], BF16, tag="ones")
                nc.sync.dma_start(ones_t[:], ones_d[:])
                cos_t = cpool.tile([128, T], F32, tag="cos")
                nc.sync.dma_start(cos_t[:], cos_d[:])
                sin_t = cpool.tile([128, T], F32, tag="sin")
                nc.sync.dma_start(sin_t[:], sin_d[:])
                cs_loaded.append((mask_t, pad_t, pt_t, ones_t, cos_t, sin_t))
                return cs_loaded[0]

            def _one_rep():
                attn_sb = [[None] * HC for _ in range(B)]

                with tc.tile_pool(name="qkr", bufs=1) as qkrpool, \
                     tc.tile_pool(name="vsb", bufs=16) as vpool, \
                     tc.tile_pool(name="khp", bufs=2) as khpool, \
                     tc.tile_pool(name="vhp", bufs=1) as vhpool, \
                     tc.tile_pool(name="expp", bufs=3) as epool, \
                     tc.tile_pool(name="smol", bufs=1) as smpool:
                    qk_rot = [qkrpool.tile([128, T], BF16, tag=f"qkr{rt}",
                                           name=f"qkr{rt}")
                              for rt in range(8)]
                    v_sb = [None] * 16  # global t-tile index

                    RT_ORDER = (0, 4, 1, 5, 2, 6, 3, 7)

                    def _vht_dma(b):
                        if not SH[b]:
                            return None
                        vht = vhpool.tile([128, SH[b], W], BF16,
                                          tag="vh", name=f"vh_t{b}")
                        nc.sync.dma_start(
                            vht[:],
                            vh_d[b][:].rearrange("(s p) c -> p s c", p=128))
                        return vht

                    def _attn_pair(b, hp, vht):
                        mask_t, pad_t, pt_t, ones_t, cos_t, sin_t = \
                            cs_loaded[0]
                        boff = b * QLEN
                        S = SH[b] + 4
                        hs = (2 * hp, 2 * hp + 1)
                        kh_t, dn, pv = {}, {}, {}
                        for h in hs:
                            if SH[b]:
                                kh_t[h] = khpool.tile(
                                    [128, hv[b]], BF16, tag="kh",
                                    name=f"kh{b}_{h}")
                                nc.sync.dma_start(
                                    kh_t[h][:],
                                    kh_d[b][h * 128:(h + 1) * 128, :])
                            dn[h] = pspool.tile(
                                [1, QLEN], F32, tag="ps", name=f"dn{b}_{h}")
                            pv[h] = pspool.tile(
                                [128, QLEN], F32, tag="ps", name=f"pv{b}_{h}")
                        for st in range(S):
                            # new-kv slab j: queries < j*128 are fully
                            # masked; restrict to cols c0..
                            if st < SH[b]:
                                c0 = 0
                            else:
                                c0 = (st - SH[b]) * 128
                            csl = slice(c0, QLEN)
                            for h in hs:
                                sc = pspool.tile([128, QLEN], F32, tag="ps")
                                if st < SH[b]:
                                    lhsT = kh_t[h][:, st * 128:(st + 1) * 128]
                                else:
                                    j = st - SH[b]
                                    lhsT = qk_rot[4 + h][
                                        :, boff + j * 128:boff + (j + 1) * 128]
                                nc.tensor.matmul(
                                    sc[:, csl], lhsT,
                                    qk_rot[h][:, boff + c0:boff + QLEN],
                                    start=True, stop=True)
                                if st == SH[b] - 1 and hist[b] != hv[b]:
                                    nc.vector.tensor_scalar_add(
                                        sc[:], sc[:], pad_t[:, b:b + 1])
                                if st >= SH[b]:
                                    nc.vector.tensor_add(
                                        sc[:, csl], sc[:, csl],
                                        mask_t[:, 384:896 - c0])
                                ex = epool.tile([128, QLEN], BF16, tag="exp")
                                nc.scalar.activation(
                                    ex[:, csl], sc[:, csl],
                                    mybir.ActivationFunctionType.Exp,
                                    scale=SCALE)
                                nc.tensor.matmul(
                                    dn[h][:, csl], ones_t[:], ex[:, csl],
                                    start=(st == 0), stop=(st == S - 1))
                                if st < SH[b]:
                                    vt = vht[:, st, h * 128:(h + 1) * 128]
                                else:
                                    gt = b * 4 + (st - SH[b])
                                    vt = v_sb[gt][:, h * 128:(h + 1) * 128]
                                nc.tensor.matmul(
                                    pv[h][:, csl], vt, ex[:, csl],
                                    start=(st == 0), stop=(st == S - 1))
                        for h in hs:
                            rc = smpool.tile([1, QLEN], F32, tag="rc")
                            nc.vector.reciprocal(rc[:], dn[h][:])
                            bcs = smpool.tile([128, QLEN], F32, tag="bcs")
                            nc.gpsimd.partition_broadcast(bcs[:], rc[:])
                            at = apool.tile([128, QLEN], BF16, tag="attn")
                            nc.vector.tensor_mul(at[:], pv[h][:], bcs[:])
                            attn_sb[b][h] = at

                    import contextlib
                    proj_stack = contextlib.ExitStack()
                    hidpool = proj_stack.enter_context(
                        tc.tile_pool(name="hid", bufs=8))
                    wqpool = proj_stack.enter_context(
                        tc.tile_pool(name="wst", bufs=2))
                    wvpool = proj_stack.enter_context(
                        tc.tile_pool(name="wvst", bufs=3))
                    rppool = proj_stack.enter_context(
                        tc.tile_pool(name="rope", bufs=2))

                    for ps in range(2):  # pass over seqs (2*ps, 2*ps+1)
                        psl = slice(ps * 1024, (ps + 1) * 1024)

                        def _hid_part(ht, kc, q, step):
                            nc.sync.dma_start(
                                ht[:, q * step:(q + 1) * step, :],
                                hiddenT_d[kc * 512 + q * step * 128:
                                          kc * 512 + (q + 1) * step * 128,
                                          psl]
                                .rearrange("(s p) t -> p s t", p=128))

                        def _hid_dma(kc, split=1):
                            ht = hidpool.tile([128, 4, 1024], BF16,
                                              tag="hid", name=f"hid{ps}_{kc}")
                            step = 4 // split
                            for q in range(split):
                                _hid_part(ht, kc, q, step)
                            return ht

                        def _wq_part(wqt, rt, q, step):
                            nc.sync.dma_start(
                                wqt[:, q * step:(q + 1) * step, :],
                                wqr_d[rt * HID + q * step * 128:
                                      rt * HID + (q + 1) * step * 128, :]
                                .rearrange("(s p) c -> p s c", p=128))

                        def _wq_dma(rt, split=1):
                            wqt = wqpool.tile([128, 32, 128], BF16, tag="wq")
                            step = 32 // split
                            for q in range(split):
                                _wq_part(wqt, rt, q, step)
                            return wqt

                        # hidden: 8 tiles of 4 k-slabs x 1024 t. On pass 0,
                        # interleave fine-grained hid0/wq0 sub-DMAs so the
                        # first matmul's deps arrive soonest.
                        if ps == 0:
                            ht0 = hidpool.tile([128, 4, 1024], BF16,
                                               tag="hid", name=f"hid{ps}_0")
                            wq_pre = wqpool.tile([128, 32, 128], BF16,
                                                 tag="wq")
                            for q in range(4):
                                _hid_part(ht0, 0, q, 1)
                                _wq_part(wq_pre, RT_ORDER[0], q, 8)
                            hid_c = [ht0]
                        else:
                            hid_c = [_hid_dma(0)]
                            wq_pre = _wq_dma(RT_ORDER[0])
                        # second weight tile lands mid hid-stream: early
                        # enough for rt pair 2, late enough not to delay
                        # the first chain's hid tiles
                        wq_pre2 = None
                        for kc in range(1, 8):
                            hid_c.append(_hid_dma(kc))
                            if kc == 3:
                                wq_pre2 = _wq_dma(RT_ORDER[1])
                        if not cs_loaded:
                            _load_cs()
                        mask_t, pad_t, pt_t, ones_t, cos_t, sin_t = \
                            cs_loaded[0]

                        # QK projection + RoPE, rt order pairs Q_h with K_h
                        # so attention for head h can start early.
                        for ri, rt in enumerate(RT_ORDER):
                            if ri == 0:
                                wqt = wq_pre
                            elif ri == 1:
                                wqt = wq_pre2
                            else:
                                wqt = _wq_dma(rt)
                            # both chunks' pq chains back-to-back so the
                            # rot matmuls never stall PE on the ACT copy
                            pqs, qss = [], []
                            for ch in range(2):  # 512-token chunks
                                tsl = slice(ch * 512, (ch + 1) * 512)
                                pq = pspool.tile([128, QLEN], F32, tag="ps")
                                for k in range(32):
                                    nc.tensor.matmul(
                                        pq[:], wqt[:, k, :],
                                        hid_c[k // 4][:, k % 4, tsl],
                                        start=(k == 0), stop=(k == 31))
                                qs = rppool.tile([128, QLEN], F32R, tag="qs")
                                nc.scalar.copy(qs[:], pq[:])
                                pqs.append(pq)
                                qss.append(qs)
                            for ch in range(2):
                                gsl = slice(ps * 1024 + ch * 512,
                                            ps * 1024 + ch * 512 + 512)
                                rot = pspool.tile([128, QLEN], F32, tag="ps")
                                nc.tensor.matmul(rot[:], pt_t[:], qss[ch][:],
                                                 start=True, stop=True)
                                t1 = rppool.tile([128, QLEN], F32, tag="t1")
                                nc.vector.tensor_mul(t1[:], rot[:],
                                                     sin_t[:, gsl])
                                t2 = rppool.tile([128, QLEN], F32, tag="t2")
                                nc.vector.tensor_mul(t2[:], qss[ch][:],
                                                     cos_t[:, gsl])
                                nc.vector.tensor_add(qk_rot[rt][:, gsl],
                                                     t1[:], t2[:])

                        # V projection: 8 t-tiles in 2 rounds of 4 psum
                        # banks; wv streamed per 2-k-slab chunk per round
                        for rnd in range(2):
                            v_ps = [pspool.tile([128, W], F32, tag="ps",
                                                name=f"vps{ps}_{rnd}_{i}")
                                    for i in range(4)]
                            for kc2 in range(8):
                                wvt = wvpool.tile([128, 4, W], BF16, tag="wv")
                                nc.sync.dma_start(
                                    wvt[:],
                                    wvT_d[kc2 * 512:(kc2 + 1) * 512, :]
                                    .rearrange("(s p) c -> p s c", p=128))
                                for s2 in range(4):
                                    k = kc2 * 4 + s2
                                    for tt in range(4):
                                        toff = rnd * 4 + tt
                                        nc.tensor.matmul(
                                            v_ps[tt][:],
                                            hid_c[k // 4][:, k % 4,
                                                          toff * 128:
                                                          (toff + 1) * 128],
                                            wvt[:, s2, :],
                                            start=(k == 0), stop=(k == 31))
                            for tt in range(4):
                                gt = ps * 8 + rnd * 4 + tt
                                vt_sb = vpool.tile([128, W], BF16, tag="vsb",
                                                   name=f"vsb{gt}")
                                nc.vector.tensor_copy(vt_sb[:], v_ps[tt][:])
                                v_sb[gt] = vt_sb

                        # attention for pass-0 seqs overlaps pass-1
                        # projection; pass-1 seqs are interleaved with
                        # o_proj blocks below
                        if ps == 0:
                            for b in (0, 1):
                                vht = _vht_dma(b)
                                for hp in range(HC // 2):
                                    _attn_pair(b, hp, vht)

                    proj_stack.close()

                    # o_proj: wo chunks resident; pass-1 attention is
                    # interleaved with ready o_proj row-blocks so PE never
                    # idles on the exp-latency chains of the tail seqs
                    with tc.tile_pool(name="wop", bufs=8) as wopool, \
                         tc.tile_pool(name="stg", bufs=3) as stpool:
                        wots = []
                        for ic in range(8):
                            isl = slice(ic * 512, (ic + 1) * 512)
                            wot = wopool.tile([128, 4, 512], BF16, tag="wo",
                                              name=f"wot{ic}")
                            nc.sync.dma_start(
                                wot[:],
                                woT_d[:, isl].rearrange("(s p) c -> p s c",
                                                        p=128))
                            wots.append(wot)

                        def _oproj_tt(tt):
                            b, q = tt // 4, tt % 4
                            st_ = stpool.tile([128, HID], BF16, tag="stg",
                                              name=f"stg{tt}")
                            for ic in range(8):
                                po = pspool.tile([128, 512], F32, tag="ps",
                                                 name=f"po{tt}_{ic}")
                                for jt in range(4):
                                    nc.tensor.matmul(
                                        po[:],
                                        attn_sb[b][jt][:,
                                                       q * 128:(q + 1) * 128],
                                        wots[ic][:, jt, :],
                                        start=(jt == 0), stop=(jt == 3))
                                nc.vector.tensor_copy(
                                    st_[:, ic * 512:(ic + 1) * 512], po[:])
                            nc.sync.dma_start(
                                out_d[tt * 128:(tt + 1) * 128, :], st_[:])

                        vht2 = _vht_dma(2)
                        _attn_pair(2, 0, vht2)
                        _oproj_tt(0)
                        _oproj_tt(1)
                        _attn_pair(2, 1, vht2)
                        _oproj_tt(2)
                        _oproj_tt(3)
                        vht3 = _vht_dma(3)
                        _attn_pair(3, 0, vht3)
                        _oproj_tt(4)
                        _oproj_tt(5)
                        _attn_pair(3, 1, vht3)
                        _oproj_tt(6)
                        _oproj_tt(7)
                        for tt in range(8, 16):
                            _oproj_tt(tt)

            for _rep in range(reps):
                _one_rep()
    nc.compile()
    return {"nc": nc}


def _get(hist, reps=1):
    if (hist, reps) not in _cache:
        _cache[(hist, reps)] = _build(hist, reps)
    return _cache[(hist, reps)]


def prepare_in_maps(inputs):
    hidden = np.asarray(inputs["hidden_states"], np.float32)
    w_pack = np.asarray(inputs["w_pack"], np.float32)
    w_o = np.asarray(inputs["w_o"], np.float32)
    kc = np.asarray(inputs["key_cache"], np.float32).reshape(NBLOCKS * BS, H, D)
    vc = np.asarray(inputs["value_cache"], np.float32).reshape(NBLOCKS * BS, H, D)
    bo = np.asarray(inputs["block_offsets"], np.int32)
    hist = tuple(int(x) for x in np.asarray(inputs["history_lengths"]))
    assert all(0 <= h and h + QLEN <= MAXBLK * BS for h in hist)
    hv = [_round128(h) for h in hist]

    built = _get(hist)
    hiddenT = np.ascontiguousarray(hidden.T).astype(BF16NP)

    in_maps = []
    for c in range(NCORES):
        rs = slice(c * W, (c + 1) * W)
        wqk = np.concatenate(
            [w_pack[rs], w_pack[HID + c * W:HID + (c + 1) * W]], axis=0)
        # wqr[rt, s, p, col] = wqk[rt*128+col, s*128+p]
        wqr = np.ascontiguousarray(
            wqk.reshape(8, 128, 32, 128).transpose(0, 2, 3, 1)
            .reshape(8 * HID, 128)).astype(BF16NP)
        wv = w_pack[2 * HID + c * W:2 * HID + (c + 1) * W]
        im = {
            "hiddenT": hiddenT,
            "wqr": wqr,
            "wvT": np.ascontiguousarray(wv.T).astype(BF16NP),
            "woT": np.ascontiguousarray(w_o[:, rs].T).astype(BF16NP),
        }
        for b in range(B):
            if not hv[b]:
                continue
            nblk = (hist[b] + BS - 1) // BS
            rows = (bo[b, :nblk, None] * BS +
                    np.arange(BS)[None, :]).reshape(-1)[:hist[b]]
            khp = np.zeros((hv[b], HC, D), np.float32)
            khp[:hist[b]] = kc[rows][:, c * HC:(c + 1) * HC, :]
            vhp = np.zeros((hv[b], HC, D), np.float32)
            vhp[:hist[b]] = vc[rows][:, c * HC:(c + 1) * HC, :]
            im[f"khT{b}"] = np.ascontiguousarray(
                khp.transpose(1, 2, 0).reshape(W, hv[b])).astype(BF16NP)
            im[f"vh{b}"] = np.ascontiguousarray(
                vhp.reshape(hv[b], W)).astype(BF16NP)
        in_maps.append(im)
    return built["nc"], in_maps


def kernel(**inputs):
    global last_results
    from concourse.bass_utils import run_bass_kernel_spmd

    nc, in_maps = prepare_in_maps(inputs)
    last_results = run_bass_kernel_spmd(nc, in_maps,
                                        core_ids=list(range(NCORES)))
    acc = np.zeros((T, HID), np.float64)
    for c in range(NCORES):
        acc += last_results.results[c]["out"].astype(np.float32)
    return acc.astype(np.float32)


# revision 34
# speedup vs baseline: 2.3093x; 2.3093x over previous
"""Baichuan paged-attention layer on 8 trn2 cores, tensor-parallel over heads.

Per core c: heads 4c..4c+3, all matmul math in bf16 (f32 PSUM accumulate).
Two passes of 2 sequences each: per pass the QK weights are loaded once and
the 1024-token hidden chunk is resident in SBUF, so QK weights are streamed
2x total (vs 4x per-seq in the f32 baseline). RoPE runs in f32 off PSUM and
rounds once to bf16. Attention (scores, exp, pv) is bf16 against [gathered
history KV + new KV]. o_proj stages [128,4096] row-blocks and writes bf16
partials; the host sums the 8 partials in f64.
"""
import sys

sys.path.insert(0, "/opt/trn_rl_repo")
import numpy as np
import ml_dtypes

BF16NP = ml_dtypes.bfloat16

H = 32; D = 128; HID = 4096; BS = 64; NBLOCKS = 128
B = 4; QLEN = 512; MAXBLK = 24; ROPE_BASE = 10000.0
T = B * QLEN; NCORES = 8; HC = H // NCORES; W = HC * D  # 4 heads, 512 wide
NEG = -1.0e30
SCALE = 1.0 / float(np.sqrt(D))

_cache = {}
last_results = None  # BassKernelResults of the most recent run (for test.py)


def _round128(x):
    return (x + 127) // 128 * 128


def _build(hist, reps=1):
    import concourse.bass as bass
    import concourse.tile as tile
    from concourse import bacc, mybir

    F32 = mybir.dt.float32
    F32R = mybir.dt.float32r
    BF16 = mybir.dt.bfloat16

    hv = [_round128(h) for h in hist]
    SH = [x // 128 for x in hv]

    nc = bacc.Bacc("TRN2", target_bir_lowering=False, debug=False,
                   num_devices=NCORES)
    hiddenT_d = nc.dram_tensor("hiddenT", [HID, T], BF16, kind="ExternalInput")
    # wqr: [rt, kslab, p, col] flattened -> [8*HID, 128]
    wqr_d = nc.dram_tensor("wqr", [8 * HID, 128], BF16, kind="ExternalInput")
    wvT_d = nc.dram_tensor("wvT", [HID, W], BF16, kind="ExternalInput")
    woT_d = nc.dram_tensor("woT", [W, HID], BF16, kind="ExternalInput")
    kh_d = [nc.dram_tensor(f"khT{b}", [W, hv[b]], BF16, kind="ExternalInput")
            if hv[b] else None for b in range(B)]
    vh_d = [nc.dram_tensor(f"vh{b}", [hv[b], W], BF16, kind="ExternalInput")
            if hv[b] else None for b in range(B)]
    out_d = nc.dram_tensor("out", [T, HID], BF16, kind="ExternalOutput")

    # host-built tables baked into the NEFF
    inv = 1.0 / (ROPE_BASE ** (np.arange(0, D, 2) / D))
    pos = np.concatenate([h + np.arange(QLEN) for h in hist]).astype(np.float64)
    ang = np.concatenate([inv, inv])[:, None] * pos[None, :]
    cos_d = nc.inline_tensor(np.cos(ang).astype(np.float32), name="cosT")
    sin_d = nc.inline_tensor(np.sin(ang).astype(np.float32), name="sinT")

    mask_np = np.where(
        np.arange(128)[:, None] <= np.arange(896)[None, :] - 384,
        0.0, NEG).astype(np.float32)
    mask_d = nc.inline_tensor(mask_np, name="maskS")

    pad_np = np.zeros((128, B), np.float32)
    for b in range(B):
        if hv[b]:
            pad_np[:, b] = np.where(hv[b] - 128 + np.arange(128) >= hist[b],
                                    NEG, 0.0)
    pad_d = nc.inline_tensor(pad_np, name="padc")

    Pm = np.zeros((128, 128), np.float32)
    for d in range(64):
        Pm[d, d + 64] = -1.0
        Pm[d + 64, d] = 1.0
    pt_d = nc.inline_tensor(np.ascontiguousarray(Pm.T), name="permT")
    ones_d = nc.inline_tensor(np.ones((128, 1), BF16NP), name="ones")

    with tile.TileContext(nc) as tc:
        with tc.tile_pool(name="const", bufs=1) as cpool, \
             tc.tile_pool(name="attn", bufs=16) as apool, \
             tc.tile_pool(name="psum", bufs=8, space="PSUM") as pspool:
            cs_loaded = []

            def _load_cs():
                # deferred so the first hid/wq DMAs win the HWDGE FIFO race
                mask_t = cpool.tile([128, 896], F32, tag="mask")
                nc.sync.dma_start(mask_t[:], mask_d[:])
                pad_t = cpool.tile([128, B], F32, tag="pad")
                nc.sync.dma_start(pad_t[:], pad_d[:])
                pt_t = cpool.tile([128, 128], F32R, tag="pt")
                nc.sync.dma_start(pt_t[:], pt_d[:].bitcast(F32R))
                ones_t = cpool.tile([128, 1], BF16, tag="ones")
                nc.sync.dma_start(ones_t[:], ones_d[:])
                cos_t = cpool.tile([128, T], F32, tag="cos")
                nc.sync.dma_start(cos_t[:], cos_d[:])
                sin_t = cpool.tile([128, T], F32, tag="sin")
                nc.sync.dma_start(sin_t[:], sin_d[:])
                cs_loaded.append((mask_t, pad_t, pt_t, ones_t, cos_t, sin_t))
                return cs_loaded[0]

            def _one_rep():
                attn_sb = [[None] * HC for _ in range(B)]

                with tc.tile_pool(name="qkr", bufs=1) as qkrpool, \
                     tc.tile_pool(name="vsb", bufs=16) as vpool, \
                     tc.tile_pool(name="khp", bufs=2) as khpool, \
                     tc.tile_pool(name="vhp", bufs=1) as vhpool, \
                     tc.tile_pool(name="expp", bufs=3) as epool, \
                     tc.tile_pool(name="smol", bufs=1) as smpool:
                    qk_rot = [qkrpool.tile([128, T], BF16, tag=f"qkr{rt}",
                                           name=f"qkr{rt}")
                              for rt in range(8)]
                    v_sb = [None] * 16  # global t-tile index

                    RT_ORDER = (0, 4, 1, 5, 2, 6, 3, 7)

                    def _vht_dma(b):
                        if not SH[b]:
                            return None
                        vht = vhpool.tile([128, SH[b], W], BF16,
                                          tag="vh", name=f"vh_t{b}")
                        nc.sync.dma_start(
                            vht[:],
                            vh_d[b][:].rearrange("(s p) c -> p s c", p=128))
                        return vht

                    def _attn_pair(b, hp, vht):
                        mask_t, pad_t, pt_t, ones_t, cos_t, sin_t = \
                            cs_loaded[0]
                        boff = b * QLEN
                        S = SH[b] + 4
                        hs = (2 * hp, 2 * hp + 1)
                        kh_t, dn, pv = {}, {}, {}
                        for h in hs:
                            if SH[b]:
                                kh_t[h] = khpool.tile(
                                    [128, hv[b]], BF16, tag="kh",
                                    name=f"kh{b}_{h}")
                                nc.sync.dma_start(
                                    kh_t[h][:],
                                    kh_d[b][h * 128:(h + 1) * 128, :])
                            dn[h] = pspool.tile(
                                [1, QLEN], F32, tag="ps", name=f"dn{b}_{h}")
                            pv[h] = pspool.tile(
                                [128, QLEN], F32, tag="ps", name=f"pv{b}_{h}")
                        for st in range(S):
                            # new-kv slab j: queries < j*128 are fully
                            # masked; restrict to cols c0..
                            if st < SH[b]:
                                c0 = 0
                            else:
                                c0 = (st - SH[b]) * 128
                            csl = slice(c0, QLEN)
                            for h in hs:
                                sc = pspool.tile([128, QLEN], F32, tag="ps")
                                if st < SH[b]:
                                    lhsT = kh_t[h][:, st * 128:(st + 1) * 128]
                                else:
                                    j = st - SH[b]
                                    lhsT = qk_rot[4 + h][
                                        :, boff + j * 128:boff + (j + 1) * 128]
                                nc.tensor.matmul(
                                    sc[:, csl], lhsT,
                                    qk_rot[h][:, boff + c0:boff + QLEN],
                                    start=True, stop=True)
                                if st == SH[b] - 1 and hist[b] != hv[b]:
                                    nc.vector.tensor_scalar_add(
                                        sc[:], sc[:], pad_t[:, b:b + 1])
                                if st >= SH[b]:
                                    nc.vector.tensor_add(
                                        sc[:, csl], sc[:, csl],
                                        mask_t[:, 384:896 - c0])
                                ex = epool.tile([128, QLEN], BF16, tag="exp")
                                nc.scalar.activation(
                                    ex[:, csl], sc[:, csl],
                                    mybir.ActivationFunctionType.Exp,
                                    scale=SCALE)
                                nc.tensor.matmul(
                                    dn[h][:, csl], ones_t[:], ex[:, csl],
                                    start=(st == 0), stop=(st == S - 1))
                                if st < SH[b]:
                                    vt = vht[:, st, h * 128:(h + 1) * 128]
                                else:
                                    gt = b * 4 + (st - SH[b])
                                    vt = v_sb[gt][:, h * 128:(h + 1) * 128]
                                nc.tensor.matmul(
                                    pv[h][:, csl], vt, ex[:, csl],
                                    start=(st == 0), stop=(st == S - 1))
                        for h in hs:
                            rc = smpool.tile([1, QLEN], F32, tag="rc")
                            nc.vector.reciprocal(rc[:], dn[h][:])
                            bcs = smpool.tile([128, QLEN], F32, tag="bcs")
                            nc.gpsimd.partition_broadcast(bcs[:], rc[:])
                            at = apool.tile([128, QLEN], BF16, tag="attn")
                            nc.vector.tensor_mul(at[:], pv[h][:], bcs[:])
                            attn_sb[b][h] = at

                    import contextlib
                    proj_stack = contextlib.ExitStack()
                    hidpool = proj_stack.enter_context(
                        tc.tile_pool(name="hid", bufs=8))
                    wqpool = proj_stack.enter_context(
                        tc.tile_pool(name="wst", bufs=2))
                    wvpool = proj_stack.enter_context(
                        tc.tile_pool(name="wvst", bufs=3))
                    rppool = proj_stack.enter_context(
                        tc.tile_pool(name="rope", bufs=2))

                    for ps in range(2):  # pass over seqs (2*ps, 2*ps+1)
                        psl = slice(ps * 1024, (ps + 1) * 1024)

                        def _hid_part(ht, kc, q, step):
                            nc.sync.dma_start(
                                ht[:, q * step:(q + 1) * step, :],
                                hiddenT_d[kc * 512 + q * step * 128:
                                          kc * 512 + (q + 1) * step * 128,
                                          psl]
                                .rearrange("(s p) t -> p s t", p=128))

                        def _hid_dma(kc, split=1):
                            ht = hidpool.tile([128, 4, 1024], BF16,
                                              tag="hid", name=f"hid{ps}_{kc}")
                            step = 4 // split
                            for q in range(split):
                                _hid_part(ht, kc, q, step)
                            return ht

                        def _wq_part(wqt, rt, q, step):
                            nc.sync.dma_start(
                                wqt[:, q * step:(q + 1) * step, :],
                                wqr_d[rt * HID + q * step * 128:
                                      rt * HID + (q + 1) * step * 128, :]
                                .rearrange("(s p) c -> p s c", p=128))

                        def _wq_dma(rt, split=1):
                            wqt = wqpool.tile([128, 32, 128], BF16, tag="wq")
                            step = 32 // split
                            for q in range(split):
                                _wq_part(wqt, rt, q, step)
                            return wqt

                        # hidden: 8 tiles of 4 k-slabs x 1024 t. On pass 0,
                        # interleave fine-grained hid0/wq0 sub-DMAs so the
                        # first matmul's deps arrive soonest.
                        if ps == 0:
                            ht0 = hidpool.tile([128, 4, 1024], BF16,
                                               tag="hid", name=f"hid{ps}_0")
                            wq_pre = wqpool.tile([128, 32, 128], BF16,
                                                 tag="wq")
                            for q in range(4):
                                _hid_part(ht0, 0, q, 1)
                                _wq_part(wq_pre, RT_ORDER[0], q, 8)
                            hid_c = [ht0]
                        else:
                            hid_c = [_hid_dma(0)]
                            wq_pre = _wq_dma(RT_ORDER[0])
                        # second weight tile lands mid hid-stream: early
                        # enough for rt pair 2, late enough not to delay
                        # the first chain's hid tiles
                        wq_pre2 = None
                        for kc in range(1, 8):
                            hid_c.append(_hid_dma(kc))
                            if kc == 3:
                                wq_pre2 = _wq_dma(RT_ORDER[1])
                        if not cs_loaded:
                            _load_cs()
                        mask_t, pad_t, pt_t, ones_t, cos_t, sin_t = \
                            cs_loaded[0]

                        # QK projection + RoPE, rt order pairs Q_h with K_h
                        # so attention for head h can start early.
                        for ri, rt in enumerate(RT_ORDER):
                            if ri == 0:
                                wqt = wq_pre
                            elif ri == 1:
                                wqt = wq_pre2
                            else:
                                wqt = _wq_dma(rt)
                            # both chunks' pq chains back-to-back so the
                            # rot matmuls never stall PE on the ACT copy
                            pqs, qss = [], []
                            for ch in range(2):  # 512-token chunks
                                tsl = slice(ch * 512, (ch + 1) * 512)
                                pq = pspool.tile([128, QLEN], F32, tag="ps")
                                for k in range(32):
                                    nc.tensor.matmul(
                                        pq[:], wqt[:, k, :],
                                        hid_c[k // 4][:, k % 4, tsl],
                                        start=(k == 0), stop=(k == 31))
                                qs = rppool.tile([128, QLEN], F32R, tag="qs")
                                nc.scalar.copy(qs[:], pq[:])
                                pqs.append(pq)
                                qss.append(qs)
                            for ch in range(2):
                                gsl = slice(ps * 1024 + ch * 512,
                                            ps * 1024 + ch * 512 + 512)
                                rot = pspool.tile([128, QLEN], F32, tag="ps")
                                nc.tensor.matmul(rot[:], pt_t[:], qss[ch][:],
                                                 start=True, stop=True)
                                t1 = rppool.tile([128, QLEN], F32, tag="t1")
                                nc.vector.tensor_mul(t1[:], rot[:],
                                                     sin_t[:, gsl])
                                t2 = rppool.tile([128, QLEN], F32, tag="t2")
                                nc.vector.tensor_mul(t2[:], qss[ch][:],
                                                     cos_t[:, gsl])
                                nc.vector.tensor_add(qk_rot[rt][:, gsl],
                                                     t1[:], t2[:])

                        # V projection: 8 t-tiles in 2 rounds of 4 psum
                        # banks; wv streamed per 2-k-slab chunk per round
                        for rnd in range(2):
                            v_ps = [pspool.tile([128, W], F32, tag="ps",
                                                name=f"vps{ps}_{rnd}_{i}")
                                    for i in range(4)]
                            for kc2 in range(8):
                                wvt = wvpool.tile([128, 4, W], BF16, tag="wv")
                                nc.sync.dma_start(
                                    wvt[:],
                                    wvT_d[kc2 * 512:(kc2 + 1) * 512, :]
                                    .rearrange("(s p) c -> p s c", p=128))
                                for s2 in range(4):
                                    k = kc2 * 4 + s2
                                    for tt in range(4):
                                        toff = rnd * 4 + tt
                                        nc.tensor.matmul(
                                            v_ps[tt][:],
                                            hid_c[k // 4][:, k % 4,
                                                          toff * 128:
                                                          (toff + 1) * 128],
                                            wvt[:, s2, :],
                                            start=(k == 0), stop=(k == 31))
                            for tt in range(4):
                                gt = ps * 8 + rnd * 4 + tt
                                vt_sb = vpool.tile([128, W], BF16, tag="vsb",
                                                   name=f"vsb{gt}")
                                nc.vector.tensor_copy(vt_sb[:], v_ps[tt][:])
                                v_sb[gt] = vt_sb

                        # attention for pass-0 seqs overlaps pass-1
                        # projection; pass-1 seqs are interleaved with
                        # o_proj blocks below
                        if ps == 0:
                            for b in (0, 1):
                                vht = _vht_dma(b)
                                for hp in range(HC // 2):
                                    _attn_pair(b, hp, vht)

                    proj_stack.close()

                    # o_proj: wo chunks resident; pass-1 attention is
                    # interleaved with ready o_proj row-blocks so PE never
                    # idles on the exp-latency chains of the tail seqs
                    with tc.tile_pool(name="wop", bufs=8) as wopool, \
                         tc.tile_pool(name="stg", bufs=3) as stpool:
                        wots = []
                        for ic in range(8):
                            isl = slice(ic * 512, (ic + 1) * 512)
                            wot = wopool.tile([128, 4, 512], BF16, tag="wo",
                                              name=f"wot{ic}")
                            nc.sync.dma_start(
                                wot[:],
                                woT_d[:, isl].rearrange("(s p) c -> p s c",
                                                        p=128))
                            wots.append(wot)

                        def _oproj_tt(tt):
                            b, q = tt // 4, tt % 4
                            st_ = stpool.tile([128, HID], BF16, tag="stg",
                                              name=f"stg{tt}")
                            for ic in range(8):
                                po = pspool.tile([128, 512], F32, tag="ps",
                                                 name=f"po{tt}_{ic}")
                                for jt in range(4):
                                    nc.tensor.matmul(
                                        po[:],
                                        attn_sb[b][jt][:,
                                                       q * 128:(q + 1) * 128],
                                        wots[ic][:, jt, :],
                                        start=(jt == 0), stop=(jt == 3))
                                nc.vector.tensor_copy(
                                    st_[:, ic * 512:(ic + 1) * 512], po[:])
                            nc.sync.dma_start(
                                out_d[tt * 128:(tt + 1) * 128, :], st_[:])

                        vht2 = _vht_dma(2)
                        _attn_pair(2, 0, vht2)
                        _oproj_tt(0)
                        _oproj_tt(1)
                        _attn_pair(2, 1, vht2)
                        _oproj_tt(2)
                        _oproj_tt(3)
                        vht3 = _vht_dma(3)
                        _attn_pair(3, 0, vht3)
                        _oproj_tt(4)
                        _oproj_tt(5)
                        _attn_pair(3, 1, vht3)
                        _oproj_tt(6)
                        _oproj_tt(7)
                        for tt in range(8, 16):
                            _oproj_tt(tt)

            for _rep in range(reps):
                _one_rep()
    nc.compile()
    return {"nc": nc}


def _get(hist, reps=1):
    if (hist, reps) not in _cache:
        _cache[(hist, reps)] = _build(hist, reps)
    return _cache[(hist, reps)]


def prepare_in_maps(inputs):
    hidden = np.asarray(inputs["hidden_states"], np.float32)
    w_pack = np.asarray(inputs["w_pack"], np.float32)
    w_o = np.asarray(inputs["w_o"], np.float32)
    kc = np.asarray(inputs["key_cache"], np.float32).reshape(NBLOCKS * BS, H, D)
    vc = np.asarray(inputs["value_cache"], np.float32).reshape(NBLOCKS * BS, H, D)
    bo = np.asarray(inputs["block_offsets"], np.int32)
    hist = tuple(int(x) for x in np.asarray(inputs["history_lengths"]))
    assert all(0 <= h and h + QLEN <= MAXBLK * BS for h in hist)
    hv = [_round128(h) for h in hist]

    built = _get(hist)
    hiddenT = np.ascontiguousarray(hidden.T).astype(BF16NP)

    in_maps = []
    for c in range(NCORES):
        rs = slice(c * W, (c + 1) * W)
        wqk = np.concatenate(
            [w_pack[rs], w_pack[HID + c * W:HID + (c + 1) * W]], axis=0)
        # wqr[rt, s, p, col] = wqk[rt*128+col, s*128+p]
        wqr = np.ascontiguousarray(
            wqk.reshape(8, 128, 32, 128).transpose(0, 2, 3, 1)
            .reshape(8 * HID, 128)).astype(BF16NP)
        wv = w_pack[2 * HID + c * W:2 * HID + (c + 1) * W]
        im = {
            "hiddenT": hiddenT,
            "wqr": wqr,
            "wvT": np.ascontiguousarray(wv.T).astype(BF16NP),
            "woT": np.ascontiguousarray(w_o[:, rs].T).astype(BF16NP),
        }
        for b in range(B):
            if not hv[b]:
                continue
            nblk = (hist[b] + BS - 1) // BS
            rows = (bo[b, :nblk, None] * BS +
                    np.arange(BS)[None, :]).reshape(-1)[:hist[b]]
            khp = np.zeros((hv[b], HC, D), np.float32)
            khp[:hist[b]] = kc[rows][:, c * HC:(c + 1) * HC, :]
            vhp = np.zeros((hv[b], HC, D), np.float32)
            vhp[:hist[b]] = vc[rows][:, c * HC:(c + 1) * HC, :]
            im[f"khT{b}"] = np.ascontiguousarray(
                khp.transpose(1, 2, 0).reshape(W, hv[b])).astype(BF16NP)
            im[f"vh{b}"] = np.ascontiguousarray(
                vhp.reshape(hv[b], W)).astype(BF16NP)
        in_maps.append(im)
    return built["nc"], in_maps


def kernel(**inputs):
    global last_results
    from concourse.bass_utils import run_bass_kernel_spmd

    nc, in_maps = prepare_in_maps(inputs)
    last_results = run_bass_kernel_spmd(nc, in_maps,
                                        core_ids=list(range(NCORES)))
    acc = np.zeros((T, HID), np.float64)
    for c in range(NCORES):
        acc += last_results.results[c]["out"].astype(np.float32)
    return acc.astype(np.float32)
